# revision 1
# baseline (speedup 1.0000x reference)
"""Dilated window attention (W=[4,8,16], R=[1,2,4]) on 8 Trainium2 NeuronCores.

Strategy (per core; b*h = 32 shards, 4 per core):
  * All three dilation groups have L=4 sub-sampled positions per window of
    w = 4*r, so every window is a 4x4 attention over d=64.
  * Unified raw-position strips: a strip is 128 consecutive sequence positions.
    For every group the scores S^T[k,q] = K.Q^T are computed over the SAME
    128x128 raw strip on TensorE (d on partitions), with a group-specific
    low-rank (1+windows) constant mask matmul accumulated into the same PSUM
    group: valid (k,q) pairs (same window, both on the group's dilation grid)
    get +0, everything else -240, which exp() turns into 0. Masks are exact in
    bf16 (-16*15 and 16*15 factorizations).
  * Q,K are cast f32->bf16 (SWDGE cast-DMA into SBUF, partition-adjacent so
    descriptors coalesce), packed bh-pair-interleaved into a DRAM scratch
    [pos, 2*64], and bulk-transposed by the DMA xbar into resident [128, 8192]
    tiles (pair member m at partitions 64m..64m+63). A PSUM accumulation group
    must keep all matmuls in one row-group range, so the mask constants are
    replicated at partition bases 0 and 64.
  * attn^T = exp(0.125*S^T) on ScalarE straight PSUM->SBUF, bf16.
  * AV + softmax denominator in one matmul: lhsT = attn^T, rhs = shared V tile
    (bf16, cast on load) with a leading ones column, so Z lands per-partition
    beside the output. rz = reciprocal(Z) * vmask_g, where vmask_g is a
    per-partition constant = w_g on the group's dilation grid and 0 elsewhere
    (w = softmax(alpha)); this both normalizes and zeroes the off-grid junk
    columns.
  * Because every group's output tile lives in the SAME raw-position partition
    layout, the three weighted contributions are summed on-chip (VectorE mult
    + GpSimd adds) and stored with ONE plain contiguous DMA. No DRAM
    accumulate pass, no dilated gathers: every DMA in the kernel is
    descriptor-coalescible (>=512B partition-adjacent chunks).
"""
import numpy as np

B, H, S, D = 2, 16, 8192, 64
N_CORES = 8
BH_PER_CORE = (B * H) // N_CORES      # 4 -> 2 bh-pairs
N_PAIRS = BH_PER_CORE // 2
RS = [1, 2, 4]                        # within-window stride per group
CHUNK = 2048                          # cast/transpose pipelining chunk (positions)
MRANK = [33, 17, 9]                   # mask rank per group (1 + windows/strip)
MOFF = [0, 33, 50]                    # row offset of each group's mask block
MTOT = 59

_CACHE = {}


def _make_masks(ml_dtypes):
    # group g: valid (k,q) iff same (4r)-window and k,q both on the r-grid.
    # mask_g = -240*J + 240*sum_c u_c u_c^T, factored exactly in bf16.
    mk_rows, mq_rows = [], []
    for r in RS:
        w = 4 * r
        nwin = 128 // w
        U = np.zeros((nwin, 128), np.float32)
        for c in range(nwin):
            U[c, w * c:w * (c + 1):r] = 1.0
        mk_rows.append(np.concatenate(
            [-16.0 * np.ones((1, 128), np.float32), 16.0 * U], 0))
        mq_rows.append(np.concatenate(
            [15.0 * np.ones((1, 128), np.float32), 15.0 * U], 0))
    mask_k = np.concatenate(mk_rows, 0)              # [59, 128]
    mask_q = np.tile(np.concatenate(mq_rows, 0), (1, 4))  # [59, 512]
    return (mask_k.astype(ml_dtypes.bfloat16), mask_q.astype(ml_dtypes.bfloat16))


def _make_vmasks(w):
    # [128, 3]: column g = w_g on partitions on the r-grid, else 0
    vm = np.zeros((128, 3), np.float32)
    for g, r in enumerate(RS):
        vm[::r, g] = w[g]
    return vm


def _build(reps=1):
    import concourse.bacc as bacc
    import concourse.tile as tile
    from concourse import mybir

    F32 = mybir.dt.float32
    BF16 = mybir.dt.bfloat16

    nc = bacc.Bacc("TRN2", target_bir_lowering=False, debug=False,
                   num_devices=N_CORES)
    q = nc.dram_tensor("q", [BH_PER_CORE, S, D], F32, kind="ExternalInput")
    k = nc.dram_tensor("k", [BH_PER_CORE, S, D], F32, kind="ExternalInput")
    v = nc.dram_tensor("v", [BH_PER_CORE, S, D], F32, kind="ExternalInput")
    mask_k = nc.dram_tensor("mask_k", [MTOT, 128], BF16, kind="ExternalInput")
    mask_q = nc.dram_tensor("mask_q", [MTOT, 512], BF16, kind="ExternalInput")
    vmasks = nc.dram_tensor("vmasks", [128, 3], F32, kind="ExternalInput")
    out = nc.dram_tensor("out", [BH_PER_CORE, S, D], F32, kind="ExternalOutput")

    scr_q = nc.dram_tensor("scr_q", [N_PAIRS, S, 128], BF16)
    scr_k = nc.dram_tensor("scr_k", [N_PAIRS, S, 128], BF16)

    with tile.TileContext(nc) as tc:
        with tc.tile_pool(name="const", bufs=1) as constp, \
             tc.tile_pool(name="cast", bufs=4) as castp, \
             tc.tile_pool(name="qt", bufs=2) as qtp, \
             tc.tile_pool(name="vaug", bufs=4) as vaugp, \
             tc.tile_pool(name="attn", bufs=6) as attnp, \
             tc.tile_pool(name="outp", bufs=6) as outp, \
             tc.tile_pool(name="rz", bufs=6) as rzp, \
             tc.tile_pool(name="stp", bufs=4, space="PSUM") as stp, \
             tc.tile_pool(name="o2p", bufs=4, space="PSUM") as o2p:

            # constants: per-group mask tiles, replicated at bases 0 and 64
            # (matmul weights must start at partition base 0/32/64)
            mks, mqs = [], []
            for g in range(3):
                mkg = constp.tile([64 + MRANK[g], 128], BF16, tag=f"mk{g}")
                mqg = constp.tile([64 + MRANK[g], 512], BF16, tag=f"mq{g}")
                for mb in (0, 64):
                    nc.sync.dma_start(
                        out=mkg[mb:mb + MRANK[g], :],
                        in_=mask_k[MOFF[g]:MOFF[g] + MRANK[g], :])
                    nc.sync.dma_start(
                        out=mqg[mb:mb + MRANK[g], :],
                        in_=mask_q[MOFF[g]:MOFF[g] + MRANK[g], :])
                mks.append(mkg)
                mqs.append(mqg)
            vm = constp.tile([128, 3], F32)
            nc.sync.dma_start(out=vm[:], in_=vmasks[:])

            # f32 -> bf16 casts, pair-interleaved in SBUF, stored to scratch
            for rep in range(reps):
                for pair in range(N_PAIRS):
                    for src, scr in ((q, scr_q), (k, scr_k)):
                        for c0 in range(0, S, CHUNK):
                            ct = castp.tile([128, CHUNK // 128, 2, 64], BF16,
                                            tag="cast")
                            for m in range(2):
                                nc.gpsimd.dma_start(
                                    out=ct[:, :, m, :],
                                    in_=src[2 * pair + m, c0:c0 + CHUNK, :]
                                        .rearrange("(j p) d -> p j d", p=128))
                            nc.sync.dma_start(
                                out=scr[pair, c0:c0 + CHUNK, :].rearrange(
                                    "(j p) (m d) -> p j m d", p=128, m=2),
                                in_=ct[:])

            for rep in range(reps):
              for pair in range(N_PAIRS):
                qt = qtp.tile([128, S], BF16, tag="qt")
                kt = qtp.tile([128, S], BF16, tag="kt")
                for src, dst in ((scr_q, qt), (scr_k, kt)):
                    for c0 in range(0, S, CHUNK):
                        nc.sync.dma_start(out=dst[:, c0:c0 + CHUNK],
                                          in_=src[pair, c0:c0 + CHUNK, :],
                                          transpose=True)

                for m in range(2):
                    bh = 2 * pair + m
                    for mt in range(S // 512):
                        p0 = 512 * mt
                        if mt % 4 == 0:
                            vaug4 = vaugp.tile([128, 16, 66], BF16, tag="vaug")
                            nc.gpsimd.dma_start(
                                out=vaug4[:, :, 1:65],
                                in_=v[bh, p0:p0 + 2048, :].rearrange(
                                    "(s p) d -> p s d", s=16, p=128))
                            nc.vector.memset(vaug4[:, :, 0], 1.0)
                        vaug = vaug4[:, 4 * (mt % 4):4 * (mt % 4) + 4, :]

                        ot = outp.tile([128, 256], F32, tag="ot")
                        otv = ot[:].rearrange("p (s d) -> p s d", d=64)
                        for g in range(3):
                            st = stp.tile([128, 512], F32, tag="st",
                                          space="PSUM")
                            nc.tensor.matmul(
                                out=st[:],
                                lhsT=mks[g][64 * m:64 * m + MRANK[g], :],
                                rhs=mqs[g][64 * m:64 * m + MRANK[g], :],
                                start=True, stop=False)
                            for s4 in range(4):
                                c0 = p0 + 128 * s4
                                nc.tensor.matmul(
                                    out=st[:, 128 * s4:128 * s4 + 128],
                                    lhsT=kt[64 * m:64 * m + 64, c0:c0 + 128],
                                    rhs=qt[64 * m:64 * m + 64, c0:c0 + 128],
                                    start=False, stop=(s4 == 3))
                            attn = attnp.tile([128, 512], BF16, tag="attn")
                            nc.scalar.activation(
                                out=attn[:], in_=st[:],
                                func=mybir.ActivationFunctionType.Exp,
                                scale=float(D) ** -0.5)
                            o2 = o2p.tile([128, 260], F32, tag="o2",
                                          space="PSUM")
                            for s4 in range(4):
                                nc.tensor.matmul(
                                    out=o2[:, 65 * s4:65 * s4 + 65],
                                    lhsT=attn[:, 128 * s4:128 * s4 + 128],
                                    rhs=vaug[:, s4, 0:65],
                                    start=True, stop=True)
                            o2v = o2[:].rearrange("p (s c) -> p s c", c=65)
                            rz = rzp.tile([128, 4], F32, tag="rz")
                            nc.vector.reciprocal(out=rz[:], in_=o2v[:, :, 0])
                            rzm = rzp.tile([128, 4], F32, tag="rzm")
                            nc.vector.tensor_scalar_mul(
                                out=rzm[:], in0=rz[:], scalar1=vm[:, g:g + 1])
                            if g == 0:
                                nc.vector.tensor_tensor(
                                    out=otv,
                                    in0=o2v[:, :, 1:65],
                                    in1=rzm[:].unsqueeze(2)
                                        .to_broadcast([128, 4, 64]),
                                    op=mybir.AluOpType.mult)
                            else:
                                tmp = outp.tile([128, 256], F32, tag="tmp")
                                nc.vector.tensor_tensor(
                                    out=tmp[:].rearrange(
                                        "p (s d) -> p s d", d=64),
                                    in0=o2v[:, :, 1:65],
                                    in1=rzm[:].unsqueeze(2)
                                        .to_broadcast([128, 4, 64]),
                                    op=mybir.AluOpType.mult)
                                adder = nc.vector if g == 1 else nc.gpsimd
                                adder.tensor_tensor(
                                    out=ot[:], in0=ot[:], in1=tmp[:],
                                    op=mybir.AluOpType.add)
                        nc.sync.dma_start(
                            out=out[bh, p0:p0 + 512, :].rearrange(
                                "(s p) d -> p s d", s=4, p=128),
                            in_=otv)
    nc.compile()
    return nc


def kernel(q, k, v, alpha, _trace=False):
    import ml_dtypes
    from concourse.bass_utils import run_bass_kernel_spmd

    q = np.ascontiguousarray(np.asarray(q, dtype=np.float32))
    k = np.ascontiguousarray(np.asarray(k, dtype=np.float32))
    v = np.ascontiguousarray(np.asarray(v, dtype=np.float32))
    alpha = np.asarray(alpha, dtype=np.float32)

    aw = np.exp(alpha - alpha.max())
    w = aw / aw.sum()

    if "nc" not in _CACHE:
        _CACHE["nc"] = _build()
    nc = _CACHE["nc"]

    mask_k, mask_q = _make_masks(ml_dtypes)
    vmasks = _make_vmasks(w)
    qr = q.reshape(B * H, S, D)
    kr = k.reshape(B * H, S, D)
    vr = v.reshape(B * H, S, D)
    in_maps = []
    for c in range(N_CORES):
        sl = slice(BH_PER_CORE * c, BH_PER_CORE * (c + 1))
        in_maps.append({
            "q": qr[sl], "k": kr[sl], "v": vr[sl],
            "mask_k": mask_k, "mask_q": mask_q, "vmasks": vmasks,
        })
    res = run_bass_kernel_spmd(nc, in_maps, core_ids=list(range(N_CORES)),
                               trace=_trace)
    outs = [res.results[c]["out"] for c in range(N_CORES)]
    full = np.concatenate(outs, axis=0).reshape(B, H, S, D)
    if _trace:
        kernel._last_results = res
    return full

